# revision 16
# baseline (speedup 1.0000x reference)
"""Trainium2 Bass kernel: 8-expert top-2 MoE (SwishFFN experts, d_model=1024,
hidden=3072, N=8192 tokens), expert-parallel across 8 NeuronCores.

Contract: kernel(**inputs) takes the FULL unsharded inputs
(x[4,2048,1024], W1[8,1024,3072], W2[8,3072,1024], Wr[1024,8]) and returns
the FULL output (out[4,2048,1024], aux_loss) matching reference().

Sharding strategy (hardcoded, per the expert-parallel hint):
  - Router (softmax/top-2/renorm + aux loss) runs on host during input
    sharding -- it IS the all-to-all dispatch decision (67 MFLOP, vs the
    206 GFLOP expert FFN which all runs on device).
  - Core e receives the tokens routed to expert e (gathered, padded to a
    fixed capacity) plus W1[e]/W2[e], and computes y = silu(x @ W1e) @ W2e
    for its tokens as two fp32r matmul chains (no on-device transposes:
    everything flows in feature-major form).
  - Host scatter-adds the per-expert outputs back with routing weights.
"""

import numpy as np

B, T, C, H, E = 4, 2048, 1024, 3072, 8
N = B * T
P = 128
CC = C // P           # 8 c-chunks
HH = H // P           # 24 h-chunks
CAP = 2176            # per-expert token capacity (multiple of 128, >= max count 2169)
NT = 512              # token tile (matmul moving dim)
LAMBDA = 0.01

_CACHE: dict = {}


TILE_SIZES = [256, 384, 512, 512, 512]


def _token_tiles(sizes=None):
    # all tiles >= 256 wide (float32r runs full-rate only at N >= 256)
    sizes = TILE_SIZES if sizes is None else sizes
    assert sum(sizes) == CAP
    tiles, off = [], 0
    for nt in sizes:
        tiles.append((off, nt))
        off += nt
    return tiles


def _build_program(repeats=1, mm_dtype="float32r", tile_sizes=None,
                   py_bufs=4, ph_bufs=2, w1_bufs=6, w2_eng="gpsimd", out_eng="sync",
                   x_eng="sync", reload_w2=False, w1_split=1):
    """Per-core Bass program: yT = (silu(x @ W1) @ W2)^T for CAP tokens.

    DRAM I/O (per core):
      xT  [CC,128,CAP]: x^T            (feature-major tokens)
      w1s [HH,128,C]:   W1 repacked so w1s[ht][:, cc*128:+128] is the
                        [c-chunk, h-chunk] stationary block
      w2  [HH,128,C]:   W2.reshape(HH,128,C) (natural layout)
      yT  [CC,128,CAP] f32: output^T
    W2 stays resident in SBUF; W1 streams per token tile; both matmuls run
    as float32r (full PE rate at N>=256, ~1e-4 relative error).
    `repeats` re-runs the whole token loop (benchmarking only).
    """
    import concourse.mybir as mybir
    from concourse import bacc
    from concourse.tile import TileContext

    dt = mybir.dt
    f32 = dt.float32
    fmm = getattr(dt, mm_dtype)
    Silu = mybir.ActivationFunctionType.Silu

    nc = bacc.Bacc(None, target_bir_lowering=False)
    xT = nc.dram_tensor("xT", [CC, P, CAP], fmm, kind="ExternalInput")
    w1s = nc.dram_tensor("w1s", [HH, P, C], fmm, kind="ExternalInput")
    w2 = nc.dram_tensor("w2", [HH, P, C], fmm, kind="ExternalInput")
    yT = nc.dram_tensor("yT", [CC, P, CAP], f32, kind="ExternalOutput")

    with TileContext(nc) as tc:
        with (
            tc.tile_pool(name="resident", bufs=1) as res_pool,
            tc.tile_pool(name="w1", bufs=w1_bufs) as w1_pool,
            tc.tile_pool(name="xb", bufs=1) as x_pool,
            tc.tile_pool(name="outb", bufs=3) as out_pool,
            tc.tile_pool(name="ph", bufs=ph_bufs, space="PSUM") as ph_pool,
            tc.tile_pool(name="py", bufs=py_bufs, space="PSUM") as py_pool,
        ):
            w2_sb = res_pool.tile([P, HH * C], fmm)
            hT = res_pool.tile([P, HH * NT], fmm)
            w2_loaded = False

            for _rep in range(repeats):
                for (off, nt) in _token_tiles(tile_sizes):
                    x_sb = x_pool.tile([P, CC * NT], fmm, tag="x")
                    x_dma = getattr(nc, x_eng).dma_start
                    x_insts = []
                    for cc in range(CC):
                        x_insts.append(x_dma(
                            x_sb[:, cc * NT: cc * NT + nt], xT[cc, :, off:off + nt]
                        ))
                    if not w2_loaded:
                        # W2 preload on the gpsimd (SW-DGE) queue in hh-major
                        # 0.5 MB chunks: mm2's first chain consumes chunks in
                        # exactly this order, so it starts before the full
                        # 12.6 MB lands (chunk-granular Tile deps)
                        w2_dma = (nc.gpsimd.dma_start if w2_eng == "gpsimd"
                                  else nc.sync.dma_start)
                        for hh in range(HH):
                            w2_dma(w2_sb[:, hh * C:(hh + 1) * C], w2[hh])
                        w2_loaded = not reload_w2
                    # mm1: hT[ht] = silu( sum_cc W1(cc,ht)^T @ xT(cc) )
                    for ht in range(HH):
                        w1t = w1_pool.tile([P, C], fmm, tag="w1")
                        if w1_split == 1:
                            nc.sync.dma_start(w1t[:], w1s[ht])
                        else:
                            step = C // w1_split
                            for j in range(w1_split):
                                nc.sync.dma_start(
                                    w1t[:, j * step:(j + 1) * step],
                                    w1s[ht, :, j * step:(j + 1) * step])
                        ps = ph_pool.tile([P, NT], f32, tag="ph")
                        for cc in range(CC):
                            nc.tensor.matmul(
                                ps[:, :nt],
                                w1t[:, cc * P:(cc + 1) * P],
                                x_sb[:, cc * NT: cc * NT + nt],
                                start=(cc == 0),
                                stop=(cc == CC - 1),
                            )
                        nc.scalar.activation(
                            hT[:, ht * NT: ht * NT + nt], ps[:, :nt], Silu
                        )
                    # mm2: yT[ct] = sum_hh W2(hh,ct)^T @ hT(hh)
                    for ct in range(CC):
                        ps = py_pool.tile([P, NT], f32, tag="py")
                        for hh in range(HH):
                            nc.tensor.matmul(
                                ps[:, :nt],
                                w2_sb[:, hh * C + ct * P: hh * C + (ct + 1) * P],
                                hT[:, hh * NT: hh * NT + nt],
                                start=(hh == 0),
                                stop=(hh == HH - 1),
                            )
                        ob = out_pool.tile([P, NT], f32, tag="ob")
                        nc.vector.tensor_copy(ob[:, :nt], ps[:, :nt])
                        out_dma = (nc.gpsimd.dma_start if out_eng == "gpsimd"
                                   else nc.sync.dma_start)
                        out_dma(yT[ct, :, off:off + nt], ob[:, :nt])

    nc.compile()
    return nc


def _get_program():
    if "nc" not in _CACHE:
        _CACHE["nc"] = _build_program()
    return _CACHE["nc"]


def _routing(xf, Wr):
    """Replicates reference routing in fp32 numpy (matches jax top_k
    tie-breaking via stable argsort on negated probs)."""
    logits = xf @ Wr
    m = logits.max(-1, keepdims=True)
    ex = np.exp(logits - m)
    probs = ex / ex.sum(-1, keepdims=True)
    order = np.argsort(-probs, axis=-1, kind="stable")
    top2 = order[:, :2]
    vals = np.take_along_axis(probs, top2, -1)
    wgt = vals / vals.sum(-1, keepdims=True)
    return probs, top2, wgt


def _silu_np(v):
    return v / (1.0 + np.exp(-v))


def _pack_w1(W1e):
    # w1s[ht, p, cc*128+q] = W1e[cc*128+p, ht*128+q]
    return np.ascontiguousarray(
        W1e.reshape(CC, P, HH, P).transpose(2, 1, 0, 3).reshape(HH, P, C)
    )


def kernel(x, W1, W2, Wr):
    from concourse.bass_utils import run_bass_kernel_spmd

    x = np.asarray(x, dtype=np.float32)
    W1 = np.asarray(W1, dtype=np.float32)
    W2 = np.asarray(W2, dtype=np.float32)
    Wr = np.asarray(Wr, dtype=np.float32)
    xf = x.reshape(-1, C)

    probs, top2, wgt = _routing(xf, Wr)

    # aux loss (exactly the reference formula, detached fi)
    counts = np.bincount(top2.ravel(), minlength=E)
    fi = (counts / float(N)).astype(np.float32)
    pi = probs.mean(axis=0, dtype=np.float32)
    aux_loss = np.float32(LAMBDA * E * np.sum(fi * pi, dtype=np.float32))

    # dispatch: per-expert gather
    rows_e, w_e = [], []
    for e in range(E):
        mask = top2 == e
        rows = np.nonzero(mask.any(axis=1))[0]
        k = mask[rows].argmax(axis=1)
        rows_e.append(rows)
        w_e.append(wgt[rows, k].astype(np.float32))

    in_maps = []
    for e in range(E):
        rows = rows_e[e]
        cnt = min(len(rows), CAP)
        xg = np.zeros((CAP, C), np.float32)
        xg[:cnt] = xf[rows[:cnt]]
        in_maps.append({
            "xT": np.ascontiguousarray(xg.T).reshape(CC, P, CAP),
            "w1s": _pack_w1(W1[e]),
            "w2": np.ascontiguousarray(W2[e]).reshape(HH, P, C),
        })

    nc = _get_program()
    try:
        res = run_bass_kernel_spmd(nc, in_maps, list(range(E))).results
    except Exception:
        res = run_bass_kernel_spmd(nc, in_maps, list(range(E))).results

    out = np.zeros((N, C), np.float32)
    for e in range(E):
        rows = rows_e[e]
        cnt = min(len(rows), CAP)
        yT = res[e]["yT"].reshape(C, CAP)
        out[rows[:cnt]] += w_e[e][:cnt, None] * yT.T[:cnt]
        if len(rows) > CAP:  # capacity overflow: exact host fallback
            extra = rows[CAP:]
            h = _silu_np(xf[extra] @ W1[e])
            out[extra] += w_e[e][CAP:, None] * (h @ W2[e])

    return out.reshape(B, T, C), aux_loss


# revision 17
# speedup vs baseline: 1.3139x; 1.3139x over previous
"""Trainium2 Bass kernel: 8-expert top-2 MoE (SwishFFN experts, d_model=1024,
hidden=3072, N=8192 tokens), expert-parallel across 8 NeuronCores.

Contract: kernel(**inputs) takes the FULL unsharded inputs
(x[4,2048,1024], W1[8,1024,3072], W2[8,3072,1024], Wr[1024,8]) and returns
the FULL output (out[4,2048,1024], aux_loss) matching reference().

Sharding strategy (hardcoded, per the expert-parallel hint):
  - Router (softmax/top-2/renorm + aux loss) runs on host during input
    sharding -- it IS the all-to-all dispatch decision (67 MFLOP, vs the
    206 GFLOP expert FFN which all runs on device).
  - Core e receives the tokens routed to expert e (gathered, padded to a
    fixed capacity) plus W1[e]/W2[e], and computes y = silu(x @ W1e) @ W2e
    for its tokens as two fp32r matmul chains (no on-device transposes:
    everything flows in feature-major form).
  - Host scatter-adds the per-expert outputs back with routing weights.
"""

import numpy as np

B, T, C, H, E = 4, 2048, 1024, 3072, 8
N = B * T
P = 128
CC = C // P           # 8 c-chunks
HH = H // P           # 24 h-chunks
CAP = 2176            # per-expert token capacity (multiple of 128, >= max count 2169)
NT = 512              # token tile (matmul moving dim)
LAMBDA = 0.01

_CACHE: dict = {}


TILE_SIZES = [256, 384, 512, 512, 512]


def _token_tiles(sizes=None):
    # all tiles >= 256 wide (float32r runs full-rate only at N >= 256)
    sizes = TILE_SIZES if sizes is None else sizes
    assert sum(sizes) == CAP
    tiles, off = [], 0
    for nt in sizes:
        tiles.append((off, nt))
        off += nt
    return tiles


def _build_program(repeats=1, mm_dtype="float32r", tile_sizes=None,
                   py_bufs=4, ph_bufs=2, w1_bufs=6, w2_eng="gpsimd", out_eng="sync",
                   x_eng="sync", reload_w2=False, w1_split=1):
    """Per-core Bass program: yT = (silu(x @ W1) @ W2)^T for CAP tokens.

    DRAM I/O (per core):
      xT  [CC,128,CAP]: x^T            (feature-major tokens)
      w1s [HH,128,C]:   W1 repacked so w1s[ht][:, cc*128:+128] is the
                        [c-chunk, h-chunk] stationary block
      w2  [HH,128,C]:   W2.reshape(HH,128,C) (natural layout)
      yT  [CC,128,CAP] f32: output^T
    W2 stays resident in SBUF; W1 streams per token tile; both matmuls run
    as float32r (full PE rate at N>=256, ~1e-4 relative error).
    `repeats` re-runs the whole token loop (benchmarking only).
    """
    import concourse.mybir as mybir
    from concourse import bacc
    from concourse.tile import TileContext

    dt = mybir.dt
    f32 = dt.float32
    fmm = getattr(dt, mm_dtype)
    Silu = mybir.ActivationFunctionType.Silu

    nc = bacc.Bacc(None, target_bir_lowering=False)
    xT = nc.dram_tensor("xT", [CC, P, CAP], fmm, kind="ExternalInput")
    w1s = nc.dram_tensor("w1s", [HH, P, C], fmm, kind="ExternalInput")
    w2 = nc.dram_tensor("w2", [HH, P, C], fmm, kind="ExternalInput")
    yT = nc.dram_tensor("yT", [CC, P, CAP], f32, kind="ExternalOutput")

    with TileContext(nc) as tc:
        with (
            tc.tile_pool(name="resident", bufs=1) as res_pool,
            tc.tile_pool(name="w1", bufs=w1_bufs) as w1_pool,
            tc.tile_pool(name="xb", bufs=1) as x_pool,
            tc.tile_pool(name="outb", bufs=3) as out_pool,
            tc.tile_pool(name="ph", bufs=ph_bufs, space="PSUM") as ph_pool,
            tc.tile_pool(name="py", bufs=py_bufs, space="PSUM") as py_pool,
        ):
            w2_sb = res_pool.tile([P, HH * C], fmm)
            hT = res_pool.tile([P, HH * NT], fmm)
            w2_loaded = False

            for _rep in range(repeats):
                for (off, nt) in _token_tiles(tile_sizes):
                    x_sb = x_pool.tile([P, CC * NT], fmm, tag="x")
                    x_dma = getattr(nc, x_eng).dma_start
                    for cc in range(CC):
                        x_dma(
                            x_sb[:, cc * NT: cc * NT + nt], xT[cc, :, off:off + nt]
                        )
                    if not w2_loaded:
                        # W2 preload on the gpsimd (SW-DGE) queue in hh-major
                        # 0.5 MB chunks: mm2's first chain consumes chunks in
                        # exactly this order, so it starts before the full
                        # 12.6 MB lands (chunk-granular Tile deps)
                        w2_dma = (nc.gpsimd.dma_start if w2_eng == "gpsimd"
                                  else nc.sync.dma_start)
                        for hh in range(HH):
                            w2_dma(w2_sb[:, hh * C:(hh + 1) * C], w2[hh])
                        w2_loaded = not reload_w2
                    # mm1: hT[ht] = silu( sum_cc W1(cc,ht)^T @ xT(cc) )
                    for ht in range(HH):
                        w1t = w1_pool.tile([P, C], fmm, tag="w1")
                        if w1_split == 1:
                            nc.sync.dma_start(w1t[:], w1s[ht])
                        else:
                            step = C // w1_split
                            for j in range(w1_split):
                                nc.sync.dma_start(
                                    w1t[:, j * step:(j + 1) * step],
                                    w1s[ht, :, j * step:(j + 1) * step])
                        ps = ph_pool.tile([P, NT], f32, tag="ph")
                        for cc in range(CC):
                            nc.tensor.matmul(
                                ps[:, :nt],
                                w1t[:, cc * P:(cc + 1) * P],
                                x_sb[:, cc * NT: cc * NT + nt],
                                start=(cc == 0),
                                stop=(cc == CC - 1),
                            )
                        nc.scalar.activation(
                            hT[:, ht * NT: ht * NT + nt], ps[:, :nt], Silu
                        )
                    # mm2: yT[ct] = sum_hh W2(hh,ct)^T @ hT(hh)
                    for ct in range(CC):
                        ps = py_pool.tile([P, NT], f32, tag="py")
                        for hh in range(HH):
                            nc.tensor.matmul(
                                ps[:, :nt],
                                w2_sb[:, hh * C + ct * P: hh * C + (ct + 1) * P],
                                hT[:, hh * NT: hh * NT + nt],
                                start=(hh == 0),
                                stop=(hh == HH - 1),
                            )
                        ob = out_pool.tile([P, NT], f32, tag="ob")
                        nc.vector.tensor_copy(ob[:, :nt], ps[:, :nt])
                        out_dma = (nc.gpsimd.dma_start if out_eng == "gpsimd"
                                   else nc.sync.dma_start)
                        out_dma(yT[ct, :, off:off + nt], ob[:, :nt])

    nc.compile()
    return nc


def _get_program():
    if "nc" not in _CACHE:
        _CACHE["nc"] = _build_program()
    return _CACHE["nc"]


def _routing(xf, Wr):
    """Replicates reference routing in fp32 numpy (matches jax top_k
    tie-breaking via stable argsort on negated probs)."""
    logits = xf @ Wr
    m = logits.max(-1, keepdims=True)
    ex = np.exp(logits - m)
    probs = ex / ex.sum(-1, keepdims=True)
    order = np.argsort(-probs, axis=-1, kind="stable")
    top2 = order[:, :2]
    vals = np.take_along_axis(probs, top2, -1)
    wgt = vals / vals.sum(-1, keepdims=True)
    return probs, top2, wgt


def _silu_np(v):
    return v / (1.0 + np.exp(-v))


def _pack_w1(W1e):
    # w1s[ht, p, cc*128+q] = W1e[cc*128+p, ht*128+q]
    return np.ascontiguousarray(
        W1e.reshape(CC, P, HH, P).transpose(2, 1, 0, 3).reshape(HH, P, C)
    )


def kernel(x, W1, W2, Wr):
    from concourse.bass_utils import run_bass_kernel_spmd

    x = np.asarray(x, dtype=np.float32)
    W1 = np.asarray(W1, dtype=np.float32)
    W2 = np.asarray(W2, dtype=np.float32)
    Wr = np.asarray(Wr, dtype=np.float32)
    xf = x.reshape(-1, C)

    probs, top2, wgt = _routing(xf, Wr)

    # aux loss (exactly the reference formula, detached fi)
    counts = np.bincount(top2.ravel(), minlength=E)
    fi = (counts / float(N)).astype(np.float32)
    pi = probs.mean(axis=0, dtype=np.float32)
    aux_loss = np.float32(LAMBDA * E * np.sum(fi * pi, dtype=np.float32))

    # dispatch: per-expert gather
    rows_e, w_e = [], []
    for e in range(E):
        mask = top2 == e
        rows = np.nonzero(mask.any(axis=1))[0]
        k = mask[rows].argmax(axis=1)
        rows_e.append(rows)
        w_e.append(wgt[rows, k].astype(np.float32))

    in_maps = []
    for e in range(E):
        rows = rows_e[e]
        cnt = min(len(rows), CAP)
        xg = np.zeros((CAP, C), np.float32)
        xg[:cnt] = xf[rows[:cnt]]
        in_maps.append({
            "xT": np.ascontiguousarray(xg.T).reshape(CC, P, CAP),
            "w1s": _pack_w1(W1[e]),
            "w2": np.ascontiguousarray(W2[e]).reshape(HH, P, C),
        })

    nc = _get_program()
    try:
        res = run_bass_kernel_spmd(nc, in_maps, list(range(E))).results
    except Exception:
        res = run_bass_kernel_spmd(nc, in_maps, list(range(E))).results

    out = np.zeros((N, C), np.float32)
    for e in range(E):
        rows = rows_e[e]
        cnt = min(len(rows), CAP)
        yT = res[e]["yT"].reshape(C, CAP)
        out[rows[:cnt]] += w_e[e][:cnt, None] * yT.T[:cnt]
        if len(rows) > CAP:  # capacity overflow: exact host fallback
            extra = rows[CAP:]
            h = _silu_np(xf[extra] @ W1[e])
            out[extra] += w_e[e][CAP:, None] * (h @ W2[e])

    return out.reshape(B, T, C), aux_loss


# revision 20
# speedup vs baseline: 1.3897x; 1.0577x over previous
"""Trainium2 Bass kernel: 8-expert top-2 MoE (SwishFFN experts, d_model=1024,
hidden=3072, N=8192 tokens), expert-parallel across 8 NeuronCores.

Contract: kernel(**inputs) takes the FULL unsharded inputs
(x[4,2048,1024], W1[8,1024,3072], W2[8,3072,1024], Wr[1024,8]) and returns
the FULL output (out[4,2048,1024], aux_loss) matching reference().

Sharding strategy (hardcoded, per the expert-parallel hint):
  - Router (softmax/top-2/renorm + aux loss) runs on host during input
    sharding -- it IS the all-to-all dispatch decision (67 MFLOP, vs the
    206 GFLOP expert FFN which all runs on device).
  - Core e receives the tokens routed to expert e (gathered, padded to a
    fixed capacity) plus W1[e]/W2[e], and computes y = silu(x @ W1e) @ W2e
    for its tokens as two fp32r matmul chains (no on-device transposes:
    everything flows in feature-major form).
  - Host scatter-adds the per-expert outputs back with routing weights.
"""

import numpy as np

B, T, C, H, E = 4, 2048, 1024, 3072, 8
N = B * T
P = 128
CC = C // P           # 8 c-chunks
HH = H // P           # 24 h-chunks
CAP = 2176            # per-expert token capacity (multiple of 128, >= max count 2169)
NT = 512              # token tile (matmul moving dim)
LAMBDA = 0.01

_CACHE: dict = {}


TILE_SIZES = [256, 384, 512, 512, 512]


def _token_tiles(sizes=None):
    # all tiles >= 256 wide (float32r runs full-rate only at N >= 256)
    sizes = TILE_SIZES if sizes is None else sizes
    assert sum(sizes) == CAP
    tiles, off = [], 0
    for nt in sizes:
        tiles.append((off, nt))
        off += nt
    return tiles


def _build_program(repeats=1, mm_dtype="float32r", tile_sizes=None,
                   py_bufs=4, ph_bufs=2, w1_bufs=6, w2_eng="gpsimd", out_eng="sync",
                   x_eng="sync", reload_w2=False, w1_split=1, w2_at=0,
                   w2_trigger_ht=None):
    """Per-core Bass program: yT = (silu(x @ W1) @ W2)^T for CAP tokens.

    DRAM I/O (per core):
      xT  [CC,128,CAP]: x^T            (feature-major tokens)
      w1s [HH,128,C]:   W1 repacked so w1s[ht][:, cc*128:+128] is the
                        [c-chunk, h-chunk] stationary block
      w2  [HH,128,C]:   W2.reshape(HH,128,C) (natural layout)
      yT  [CC,128,CAP] f32: output^T
    W2 stays resident in SBUF; W1 streams per token tile; both matmuls run
    as float32r (full PE rate at N>=256, ~1e-4 relative error).
    `repeats` re-runs the whole token loop (benchmarking only).
    """
    import concourse.mybir as mybir
    from concourse import bacc
    from concourse.tile import TileContext

    dt = mybir.dt
    f32 = dt.float32
    fmm = getattr(dt, mm_dtype)
    Silu = mybir.ActivationFunctionType.Silu

    nc = bacc.Bacc(None, target_bir_lowering=False)
    xT = nc.dram_tensor("xT", [CC, P, CAP], fmm, kind="ExternalInput")
    w1s = nc.dram_tensor("w1s", [HH, P, C], fmm, kind="ExternalInput")
    w2 = nc.dram_tensor("w2", [HH, P, C], fmm, kind="ExternalInput")
    yT = nc.dram_tensor("yT", [CC, P, CAP], f32, kind="ExternalOutput")

    with TileContext(nc) as tc:
        with (
            tc.tile_pool(name="resident", bufs=1) as res_pool,
            tc.tile_pool(name="w1", bufs=w1_bufs) as w1_pool,
            tc.tile_pool(name="xb", bufs=1) as x_pool,
            tc.tile_pool(name="outb", bufs=3) as out_pool,
            tc.tile_pool(name="ph", bufs=ph_bufs, space="PSUM") as ph_pool,
            tc.tile_pool(name="py", bufs=py_bufs, space="PSUM") as py_pool,
        ):
            w2_sb = res_pool.tile([P, HH * C], fmm)
            hT = res_pool.tile([P, HH * NT], fmm)
            w2_loaded = False

            for _rep in range(repeats):
                for _ti, (off, nt) in enumerate(_token_tiles(tile_sizes)):
                    x_sb = x_pool.tile([P, CC * NT], fmm, tag="x")
                    x_dma = getattr(nc, x_eng).dma_start
                    for cc in range(CC):
                        x_dma(
                            x_sb[:, cc * NT: cc * NT + nt], xT[cc, :, off:off + nt]
                        )
                    if not w2_loaded and w2_at == 0 and w2_trigger_ht is None:
                        # W2 preload on the gpsimd (SW-DGE) queue in hh-major
                        # 0.5 MB chunks: mm2's first chain consumes chunks in
                        # exactly this order, so it starts before the full
                        # 12.6 MB lands (chunk-granular Tile deps)
                        w2_dma = (nc.gpsimd.dma_start if w2_eng == "gpsimd"
                                  else nc.sync.dma_start)
                        for hh in range(HH):
                            w2_dma(w2_sb[:, hh * C:(hh + 1) * C], w2[hh])
                        w2_loaded = not reload_w2
                    # mm1: hT[ht] = silu( sum_cc W1(cc,ht)^T @ xT(cc) )
                    for ht in range(HH):
                        w1t = w1_pool.tile([P, C], fmm, tag="w1")
                        if w1_split == 1:
                            nc.sync.dma_start(w1t[:], w1s[ht])
                        else:
                            step = C // w1_split
                            for j in range(w1_split):
                                nc.sync.dma_start(
                                    w1t[:, j * step:(j + 1) * step],
                                    w1s[ht, :, j * step:(j + 1) * step])
                        ps = ph_pool.tile([P, NT], f32, tag="ph")
                        for cc in range(CC):
                            nc.tensor.matmul(
                                ps[:, :nt],
                                w1t[:, cc * P:(cc + 1) * P],
                                x_sb[:, cc * NT: cc * NT + nt],
                                start=(cc == 0),
                                stop=(cc == CC - 1),
                            )
                        act = nc.scalar.activation(
                            hT[:, ht * NT: ht * NT + nt], ps[:, :nt], Silu
                        )
                        if (not w2_loaded and w2_trigger_ht is not None
                                and _ti == 0 and ht == w2_trigger_ht):
                            # delay the W2 preload until tile 0's mm1 is this
                            # far along, so it never starves W1(0)/x(0); the
                            # sync dep is the only way to delay it (gpsimd's
                            # stream has no other work)
                            from concourse.tile import add_dep_helper
                            w2_dma = (nc.gpsimd.dma_start if w2_eng == "gpsimd"
                                      else nc.sync.dma_start)
                            for hh in range(HH):
                                wi = w2_dma(w2_sb[:, hh * C:(hh + 1) * C], w2[hh])
                                if hh == 0:
                                    add_dep_helper(act.ins, wi.ins, sync=True,
                                                   reason="delay w2 preload")
                            w2_loaded = not reload_w2
                    if not w2_loaded:
                        # mid placement: issue W2 after tile 0's W1 stream so
                        # it never contends with mm1(0)'s inputs; mm2(0)'s
                        # first chain then consumes chunks as they land
                        w2_dma = (nc.gpsimd.dma_start if w2_eng == "gpsimd"
                                  else nc.sync.dma_start)
                        for hh in range(HH):
                            w2_dma(w2_sb[:, hh * C:(hh + 1) * C], w2[hh])
                        w2_loaded = not reload_w2
                    # mm2: yT[ct] = sum_hh W2(hh,ct)^T @ hT(hh)
                    for ct in range(CC):
                        ps = py_pool.tile([P, NT], f32, tag="py")
                        for hh in range(HH):
                            nc.tensor.matmul(
                                ps[:, :nt],
                                w2_sb[:, hh * C + ct * P: hh * C + (ct + 1) * P],
                                hT[:, hh * NT: hh * NT + nt],
                                start=(hh == 0),
                                stop=(hh == HH - 1),
                            )
                        ob = out_pool.tile([P, NT], f32, tag="ob")
                        nc.vector.tensor_copy(ob[:, :nt], ps[:, :nt])
                        out_dma = (nc.gpsimd.dma_start if out_eng == "gpsimd"
                                   else nc.sync.dma_start)
                        out_dma(yT[ct, :, off:off + nt], ob[:, :nt])

    nc.compile()
    return nc


def _get_program():
    if "nc" not in _CACHE:
        _CACHE["nc"] = _build_program()
    return _CACHE["nc"]


def _routing(xf, Wr):
    """Replicates reference routing in fp32 numpy (matches jax top_k
    tie-breaking via stable argsort on negated probs)."""
    logits = xf @ Wr
    m = logits.max(-1, keepdims=True)
    ex = np.exp(logits - m)
    probs = ex / ex.sum(-1, keepdims=True)
    order = np.argsort(-probs, axis=-1, kind="stable")
    top2 = order[:, :2]
    vals = np.take_along_axis(probs, top2, -1)
    wgt = vals / vals.sum(-1, keepdims=True)
    return probs, top2, wgt


def _silu_np(v):
    return v / (1.0 + np.exp(-v))


def _pack_w1(W1e):
    # w1s[ht, p, cc*128+q] = W1e[cc*128+p, ht*128+q]
    return np.ascontiguousarray(
        W1e.reshape(CC, P, HH, P).transpose(2, 1, 0, 3).reshape(HH, P, C)
    )


def kernel(x, W1, W2, Wr):
    from concourse.bass_utils import run_bass_kernel_spmd

    x = np.asarray(x, dtype=np.float32)
    W1 = np.asarray(W1, dtype=np.float32)
    W2 = np.asarray(W2, dtype=np.float32)
    Wr = np.asarray(Wr, dtype=np.float32)
    xf = x.reshape(-1, C)

    probs, top2, wgt = _routing(xf, Wr)

    # aux loss (exactly the reference formula, detached fi)
    counts = np.bincount(top2.ravel(), minlength=E)
    fi = (counts / float(N)).astype(np.float32)
    pi = probs.mean(axis=0, dtype=np.float32)
    aux_loss = np.float32(LAMBDA * E * np.sum(fi * pi, dtype=np.float32))

    # dispatch: per-expert gather
    rows_e, w_e = [], []
    for e in range(E):
        mask = top2 == e
        rows = np.nonzero(mask.any(axis=1))[0]
        k = mask[rows].argmax(axis=1)
        rows_e.append(rows)
        w_e.append(wgt[rows, k].astype(np.float32))

    in_maps = []
    for e in range(E):
        rows = rows_e[e]
        cnt = min(len(rows), CAP)
        xg = np.zeros((CAP, C), np.float32)
        xg[:cnt] = xf[rows[:cnt]]
        in_maps.append({
            "xT": np.ascontiguousarray(xg.T).reshape(CC, P, CAP),
            "w1s": _pack_w1(W1[e]),
            "w2": np.ascontiguousarray(W2[e]).reshape(HH, P, C),
        })

    nc = _get_program()
    try:
        res = run_bass_kernel_spmd(nc, in_maps, list(range(E))).results
    except Exception:
        res = run_bass_kernel_spmd(nc, in_maps, list(range(E))).results

    out = np.zeros((N, C), np.float32)
    for e in range(E):
        rows = rows_e[e]
        cnt = min(len(rows), CAP)
        yT = res[e]["yT"].reshape(C, CAP)
        out[rows[:cnt]] += w_e[e][:cnt, None] * yT.T[:cnt]
        if len(rows) > CAP:  # capacity overflow: exact host fallback
            extra = rows[CAP:]
            h = _silu_np(xf[extra] @ W1[e])
            out[extra] += w_e[e][CAP:, None] * (h @ W2[e])

    return out.reshape(B, T, C), aux_loss
